# revision 13
# baseline (speedup 1.0000x reference)
"""BaselineLSTM forward on 8 TRN2 NeuronCores.

Strategy (data-parallel over batch, per sharding hint):
  - batch B=64 sharded 8 ways -> 8 rows/core; W/b replicated.
  - Phase 1 (per core): xw[t*8+b, :] = x_t,b @ Wx.T + b   (f32r/bf16 matmul,
    fp32 accum), written to DRAM. Time-major row layout so each step's
    xw slice is one contiguous [8, 4096] block.
  - Phase 2: 512 sequential LSTM steps. Per step: gates = xw_t + h @ Wh.T
    via PE (stationary = h.T chunks [128,8], moving = Wh.T [128,512];
    identity-matmul accumulates xw_t into the same PSUM banks), sigmoid on
    all 4096 gate cols (g pre-scaled by 2 so tanh(g)=2*sigmoid(2g)-1),
    cell/hidden update on DVE, h transposed back via PE for the next step.

Self-contained: hardcodes shapes; host side only shards/permutes inputs and
gathers outputs.
"""

import numpy as np

B, S, D, H = 64, 512, 1024, 1024
NCORES = 8
BL = B // NCORES          # 8 batch rows per core
G4 = 4 * H                # 4096 gate columns
ND = D // 128             # 8 contraction chunks (D)
NH = H // 128             # 8 contraction chunks (H)
NG = G4 // 512            # 8 psum banks of 512 gate cols

_BUILD_CACHE = {}


def _build(T=S, mode="f32r", p1_unroll=8, p2_unroll=32, use_for_i=True):
    from concourse import bacc, mybir, tile

    dt = mybir.dt
    f32 = dt.float32
    use_bf16 = mode == "bf16"
    # matmul-operand dtype: float32r (fp32 storage, full-rate PE) or bf16.
    # float32r tensors must be *typed* f32r end-to-end (verifier requires
    # producers to round), so weights/x/xw/hT all carry wdt.
    wdt = dt.bfloat16 if use_bf16 else dt.float32r

    def mmcast(ap):
        return ap

    nc = bacc.Bacc("TRN2", target_bir_lowering=False, debug=False,
                   num_devices=NCORES)

    xt2 = nc.dram_tensor("xt2", [D, S * BL], wdt, kind="ExternalInput")
    wxt = nc.dram_tensor("wxt", [D, G4], wdt, kind="ExternalInput")
    wht = nc.dram_tensor("wht", [H, G4], wdt, kind="ExternalInput")
    biasb = nc.dram_tensor("biasb", [128, G4], f32, kind="ExternalInput")
    ident = nc.dram_tensor("ident", [BL, BL], wdt, kind="ExternalInput")
    identt = nc.dram_tensor("identt", [BL, BL], f32, kind="ExternalInput")
    out_h = nc.dram_tensor("out_h", [S * BL, H], f32, kind="ExternalOutput")
    out_c = nc.dram_tensor("out_c", [BL, H], f32, kind="ExternalOutput")
    xw = nc.dram_tensor("xw", [S * BL, G4], wdt)

    # persistent sbuf state (gate block order is [g, i, f, o], see _prep)
    ident_sb = nc.alloc_sbuf_tensor("ident_sb", [BL, BL], wdt)
    identt_sb = nc.alloc_sbuf_tensor("identt_sb", [BL, BL], f32)
    c_st = [nc.alloc_sbuf_tensor(f"c{p}", [BL, H], f32) for p in range(2)]
    hT_st = [nc.alloc_sbuf_tensor(f"hT{p}", [128, NH * BL], wdt) for p in range(2)]
    sig_sb = nc.alloc_sbuf_tensor("sig", [BL, 3 * H], f32)
    h_sb = nc.alloc_sbuf_tensor("hsb", [BL, H], f32)
    tg_sb = nc.alloc_sbuf_tensor("tg", [BL, H], f32)
    m2_sb = nc.alloc_sbuf_tensor("m2", [BL, H], f32)
    th_sb = nc.alloc_sbuf_tensor("th", [BL, H], f32)

    with tile.TileContext(nc) as tc:
        nc.sync.dma_start(out=ident_sb[:, :], in_=ident[:, :])
        nc.sync.dma_start(out=identt_sb[:, :], in_=identt[:, :])
        nc.vector.memset(c_st[0][:, :], 0.0)
        # f32r memset fails the ISA check; zero the raw bits instead
        nc.vector.memset(hT_st[0][:, :].bitcast(dt.uint32), 0)

        # ---------------- Phase 1: xw = x @ Wx.T + b ----------------
        with (
            tc.tile_pool(name="wx_pool", bufs=1) as wx_pool,
            tc.tile_pool(name="bias_pool", bufs=1) as bias_pool,
            tc.tile_pool(name="xt_pool", bufs=3) as xt_pool,
            tc.tile_pool(name="p1ps", bufs=4, space="PSUM") as p1ps,
            tc.tile_pool(name="ev_pool", bufs=3) as ev_pool,
        ):
            wx_sb = []
            for d in range(ND):
                w = wx_pool.tile([128, G4], wdt, tag=f"wx{d}")
                nc.sync.dma_start(out=w[:, :], in_=wxt[128 * d:128 * (d + 1), :])
                wx_sb.append(w)
            bias_sb = bias_pool.tile([128, G4], f32, tag="bias")
            nc.sync.dma_start(out=bias_sb[:, :], in_=biasb[:, :])

            n_rt = (T * BL) // 128          # row tiles of 128
            p1_unroll = min(p1_unroll, n_rt)
            assert n_rt % p1_unroll == 0
            from concourse import bass as _bass

            def p1_body(rt0):
                for u in range(p1_unroll):
                    rt = rt0 + u
                    xts = []
                    for d in range(ND):
                        xtile = xt_pool.tile([128, 128], wdt, tag=f"xt{d}")
                        nc.gpsimd.dma_start(
                            out=xtile[:, :],
                            in_=xt2[128 * d:128 * (d + 1), _bass.ts(rt, 128)],
                        )
                        xts.append(xtile)
                    for n in range(NG):
                        ps = p1ps.tile([128, 512], f32, tag="ps")
                        for d in range(ND):
                            nc.tensor.matmul(
                                ps[:, :],
                                mmcast(xts[d][:, :]),
                                mmcast(wx_sb[d][:, 512 * n:512 * (n + 1)]),
                                start=(d == 0),
                                stop=(d == ND - 1),
                            )
                        ev = ev_pool.tile([128, 512], wdt, tag="ev")
                        nc.vector.tensor_tensor(
                            ev[:, :], ps[:, :],
                            bias_sb[:, 512 * n:512 * (n + 1)],
                            mybir.AluOpType.add,
                        )
                        nc.gpsimd.dma_start(
                            out=xw[_bass.ts(rt, 128), 512 * n:512 * (n + 1)],
                            in_=ev[:, :],
                        )

            if use_for_i:
                with tc.For_i(0, n_rt, p1_unroll,
                              hint_engines=(mybir.EngineType.PE,)) as rt0:
                    p1_body(rt0)
            else:
                for rt0 in range(0, n_rt, p1_unroll):
                    p1_body(rt0)

        # ---------------- Phase 2: recurrence ----------------
        with (
            tc.tile_pool(name="wh_pool", bufs=1) as wh_pool,
            tc.tile_pool(name="xw_pool", bufs=2) as xw_pool,
            tc.tile_pool(name="gps", bufs=3, space="PSUM") as gps,
            tc.tile_pool(name="trps", bufs=2, space="PSUM") as trps,
        ):
            wh_sb = []
            for d in range(NH):
                w = wh_pool.tile([128, G4], wdt, tag=f"wh{d}")
                nc.sync.dma_start(out=w[:, :], in_=wht[128 * d:128 * (d + 1), :])
                wh_sb.append(w)

            from concourse import bass as _bass

            def step(t_iv, par):
                # gate block order [g, i, f, o]; one 2-bank psum tile per
                # block so sigmoid is a single [8,1024] ACT op that overlaps
                # the next block's matmuls.
                src_hT, dst_hT = hT_st[par], hT_st[1 - par]
                src_c, dst_c = c_st[par], c_st[1 - par]
                AL = mybir.AluOpType
                SIG = mybir.ActivationFunctionType.Sigmoid
                xw_t = xw_pool.tile([BL, G4], wdt, tag="xwt")
                nc.gpsimd.dma_start(out=xw_t[:, :], in_=xw[_bass.ts(t_iv, BL), :])
                ps_g = None
                for gb in range(4):          # g, i, f, o
                    ps = gps.tile([BL, 1024], f32, tag="gate_ps")
                    for half in range(2):
                        sl = slice(512 * half, 512 * (half + 1))
                        csl = slice(1024 * gb + 512 * half,
                                    1024 * gb + 512 * half + 512)
                        nc.tensor.matmul(ps[:, sl], ident_sb[:, :],
                                         xw_t[:, csl], start=True, stop=False)
                        for d in range(NH):
                            nc.tensor.matmul(
                                ps[:, sl],
                                src_hT[:, BL * d:BL * (d + 1)],
                                wh_sb[d][:, csl],
                                start=False, stop=(d == NH - 1),
                            )
                    if gb == 0:
                        # tanh(g) = 2*sigmoid(2g)-1; g pre-scaled by 2
                        nc.scalar.activation(ps[:, :], ps[:, :], SIG)
                        nc.vector.tensor_scalar(tg_sb[:, :], ps[:, :],
                                                2.0, -1.0, AL.mult, AL.add)
                        ps_g = ps
                    else:
                        # i/f/o: sigmoid psum -> sbuf slice
                        nc.scalar.activation(
                            sig_sb[:, 1024 * (gb - 1):1024 * gb], ps[:, :], SIG)
                sg_i = sig_sb[:, 0:H]
                sg_f = sig_sb[:, H:2 * H]
                sg_o = sig_sb[:, 2 * H:3 * H]
                nc.vector.tensor_tensor(tg_sb[:, :], sg_i, tg_sb[:, :], AL.mult)
                nc.vector.tensor_tensor(m2_sb[:, :], sg_f, src_c[:, :], AL.mult)
                nc.vector.tensor_tensor(dst_c[:, :], tg_sb[:, :], m2_sb[:, :],
                                        AL.add)
                nc.scalar.activation(th_sb[:, :], dst_c[:, :], SIG, scale=2.0)
                nc.vector.tensor_scalar(th_sb[:, :], th_sb[:, :], 2.0, -1.0,
                                        AL.mult, AL.add)
                nc.vector.tensor_tensor(h_sb[:, :], sg_o, th_sb[:, :], AL.mult)
                nc.gpsimd.dma_start(out=out_h[_bass.ts(t_iv, BL), :], in_=h_sb[:, :])
                for d in range(NH):
                    trp = trps.tile([128, BL], f32, tag="trp")
                    nc.tensor.transpose(
                        trp[:, :], h_sb[:, 128 * d:128 * (d + 1)], identt_sb[:, :]
                    )
                    nc.vector.tensor_copy(dst_hT[:, BL * d:BL * (d + 1)], trp[:, :])

            p2_unroll = min(p2_unroll, T)
            assert T % p2_unroll == 0 and p2_unroll % 2 == 0
            if use_for_i:
                with tc.For_i(0, T, p2_unroll,
                              hint_engines=(mybir.EngineType.PE,)) as t0:
                    for u in range(p2_unroll):
                        step(t0 + u, u % 2)
            else:
                for t0 in range(0, T, p2_unroll):
                    for u in range(p2_unroll):
                        step(t0 + u, u % 2)

            nc.sync.dma_start(out=out_c[:, :], in_=c_st[0][:, :])

    nc.compile()
    return nc


def _prep_core_inputs(x, W, b, mode="f32r"):
    """Host-side shard/permute. Returns list of 8 input dicts."""
    import ml_dtypes
    wdt = ml_dtypes.bfloat16 if mode == "bf16" else np.float32
    # reorder gate blocks i,f,o,g -> g,i,f,o and pre-scale g rows by 2
    # (tanh(g) computed as 2*sigmoid(2g)-1 on device)
    perm = np.concatenate([np.arange(3 * H, 4 * H), np.arange(0, 3 * H)])
    Wp = W[perm]
    bp = b[perm]
    scale = np.ones((G4,), np.float32)
    scale[:H] = 2.0
    Wx = Wp[:, :D]
    Wh = Wp[:, D:]
    wxt = np.ascontiguousarray((Wx * scale[:, None]).T).astype(wdt)   # [D, 4H]
    wht = np.ascontiguousarray((Wh * scale[:, None]).T).astype(wdt)   # [H, 4H]
    biasb = np.broadcast_to((bp * scale)[None, :], (128, G4)).astype(np.float32)
    biasb = np.ascontiguousarray(biasb)
    ident = np.eye(BL, dtype=wdt)
    identt = np.eye(BL, dtype=np.float32)
    maps = []
    for c in range(NCORES):
        xc = x[BL * c:BL * (c + 1)]                  # [8, S, D]
        xt2 = np.ascontiguousarray(xc.transpose(2, 1, 0).reshape(D, S * BL))
        maps.append({
            "xt2": xt2.astype(wdt),
            "wxt": wxt, "wht": wht, "biasb": biasb, "ident": ident,
            "identt": identt,
        })
    return maps


def _gather_outputs(results):
    outs = np.zeros((B, S, H), np.float32)
    c_fin = np.zeros((B, H), np.float32)
    for c, r in enumerate(results):
        oh = r["out_h"].reshape(S, BL, H).transpose(1, 0, 2)   # [BL, S, H]
        outs[BL * c:BL * (c + 1)] = oh
        c_fin[BL * c:BL * (c + 1)] = r["out_c"]
    h_fin = outs[:, -1, :].copy()
    return outs, (h_fin, c_fin)


def kernel(x, W, b, mode="f32r", T=S, use_for_i=True, _trace=False):
    x = np.asarray(x, np.float32)
    W = np.asarray(W, np.float32)
    b = np.asarray(b, np.float32)
    key = (T, mode, use_for_i)
    if key not in _BUILD_CACHE:
        _BUILD_CACHE[key] = _build(T=T, mode=mode, use_for_i=use_for_i)
    nc = _BUILD_CACHE[key]
    from concourse.bass_utils import run_bass_kernel_spmd
    in_maps = _prep_core_inputs(x, W, b, mode=mode)
    res = run_bass_kernel_spmd(nc, in_maps, list(range(NCORES)), trace=_trace)
    out = _gather_outputs(res.results)
    if _trace:
        return out, res
    return out


# revision 14
# speedup vs baseline: 1.0642x; 1.0642x over previous
"""BaselineLSTM forward on 8 TRN2 NeuronCores.

Strategy (data-parallel over batch, per sharding hint):
  - batch B=64 sharded 8 ways -> 8 rows/core; W/b replicated.
  - Phase 1 (per core): xw[t*8+b, :] = x_t,b @ Wx.T + b   (f32r/bf16 matmul,
    fp32 accum), written to DRAM. Time-major row layout so each step's
    xw slice is one contiguous [8, 4096] block.
  - Phase 2: 512 sequential LSTM steps. Per step: gates = xw_t + h @ Wh.T
    via PE (stationary = h.T chunks [128,8], moving = Wh.T [128,512];
    identity-matmul accumulates xw_t into the same PSUM banks), sigmoid on
    all 4096 gate cols (g pre-scaled by 2 so tanh(g)=2*sigmoid(2g)-1),
    cell/hidden update on DVE, h transposed back via PE for the next step.

Self-contained: hardcodes shapes; host side only shards/permutes inputs and
gathers outputs.
"""

import numpy as np

B, S, D, H = 64, 512, 1024, 1024
NCORES = 8
BL = B // NCORES          # 8 batch rows per core
G4 = 4 * H                # 4096 gate columns
ND = D // 128             # 8 contraction chunks (D)
NH = H // 128             # 8 contraction chunks (H)
NG = G4 // 512            # 8 psum banks of 512 gate cols

_BUILD_CACHE = {}


def _build(T=S, mode="f32r", p1_unroll=8, p2_unroll=32, use_for_i=True):
    from concourse import bacc, mybir, tile

    dt = mybir.dt
    f32 = dt.float32
    use_bf16 = mode == "bf16"
    # matmul-operand dtype: float32r (fp32 storage, full-rate PE) or bf16.
    # float32r tensors must be *typed* f32r end-to-end (verifier requires
    # producers to round), so weights/x/xw/hT all carry wdt.
    wdt = dt.bfloat16 if use_bf16 else dt.float32r

    def mmcast(ap):
        return ap

    nc = bacc.Bacc("TRN2", target_bir_lowering=False, debug=False,
                   num_devices=NCORES)

    xt2 = nc.dram_tensor("xt2", [D, S * BL], wdt, kind="ExternalInput")
    wxt = nc.dram_tensor("wxt", [D, G4], wdt, kind="ExternalInput")
    wht = nc.dram_tensor("wht", [H, G4], wdt, kind="ExternalInput")
    biasb = nc.dram_tensor("biasb", [128, G4], f32, kind="ExternalInput")
    ident = nc.dram_tensor("ident", [BL, BL], wdt, kind="ExternalInput")
    identt = nc.dram_tensor("identt", [BL, BL], f32, kind="ExternalInput")
    out_h = nc.dram_tensor("out_h", [S * BL, H], f32, kind="ExternalOutput")
    out_c = nc.dram_tensor("out_c", [BL, H], f32, kind="ExternalOutput")
    xw = nc.dram_tensor("xw", [S * BL, G4], wdt)

    # persistent sbuf state (gate block order is [g, i, f, o], see _prep)
    ident_sb = nc.alloc_sbuf_tensor("ident_sb", [BL, BL], wdt)
    identt_sb = nc.alloc_sbuf_tensor("identt_sb", [BL, BL], f32)
    c_st = [nc.alloc_sbuf_tensor(f"c{p}", [BL, H], f32) for p in range(2)]
    hT_st = [nc.alloc_sbuf_tensor(f"hT{p}", [128, NH * BL], wdt) for p in range(2)]
    sig_sb = nc.alloc_sbuf_tensor("sig", [BL, 3 * H], f32)
    h_sb = nc.alloc_sbuf_tensor("hsb", [BL, H], f32)
    tg_sb = nc.alloc_sbuf_tensor("tg", [BL, H], f32)
    m2_sb = nc.alloc_sbuf_tensor("m2", [BL, H], f32)
    th_sb = nc.alloc_sbuf_tensor("th", [BL, H], f32)

    with tile.TileContext(nc) as tc:
        nc.sync.dma_start(out=ident_sb[:, :], in_=ident[:, :])
        nc.sync.dma_start(out=identt_sb[:, :], in_=identt[:, :])
        nc.vector.memset(c_st[0][:, :], 0.0)
        # f32r memset fails the ISA check; zero the raw bits instead
        nc.vector.memset(hT_st[0][:, :].bitcast(dt.uint32), 0)

        # ---------------- Phase 1: xw = x @ Wx.T + b ----------------
        with (
            tc.tile_pool(name="wx_pool", bufs=1) as wx_pool,
            tc.tile_pool(name="bias_pool", bufs=1) as bias_pool,
            tc.tile_pool(name="xt_pool", bufs=3) as xt_pool,
            tc.tile_pool(name="p1ps", bufs=4, space="PSUM") as p1ps,
            tc.tile_pool(name="ev_pool", bufs=3) as ev_pool,
        ):
            wx_sb = []
            for d in range(ND):
                w = wx_pool.tile([128, G4], wdt, tag=f"wx{d}")
                nc.sync.dma_start(out=w[:, :], in_=wxt[128 * d:128 * (d + 1), :])
                wx_sb.append(w)
            bias_sb = bias_pool.tile([128, G4], f32, tag="bias")
            nc.sync.dma_start(out=bias_sb[:, :], in_=biasb[:, :])

            n_rt = (T * BL) // 128          # row tiles of 128
            p1_unroll = min(p1_unroll, n_rt)
            assert n_rt % p1_unroll == 0
            from concourse import bass as _bass

            def p1_body(rt0):
                for u in range(p1_unroll):
                    rt = rt0 + u
                    xts = []
                    for d in range(ND):
                        xtile = xt_pool.tile([128, 128], wdt, tag=f"xt{d}")
                        nc.gpsimd.dma_start(
                            out=xtile[:, :],
                            in_=xt2[128 * d:128 * (d + 1), _bass.ts(rt, 128)],
                        )
                        xts.append(xtile)
                    for n in range(NG):
                        ps = p1ps.tile([128, 512], f32, tag="ps")
                        for d in range(ND):
                            nc.tensor.matmul(
                                ps[:, :],
                                mmcast(xts[d][:, :]),
                                mmcast(wx_sb[d][:, 512 * n:512 * (n + 1)]),
                                start=(d == 0),
                                stop=(d == ND - 1),
                            )
                        ev = ev_pool.tile([128, 512], wdt, tag="ev")
                        nc.vector.tensor_tensor(
                            ev[:, :], ps[:, :],
                            bias_sb[:, 512 * n:512 * (n + 1)],
                            mybir.AluOpType.add,
                        )
                        nc.gpsimd.dma_start(
                            out=xw[_bass.ts(rt, 128), 512 * n:512 * (n + 1)],
                            in_=ev[:, :],
                        )

            if use_for_i:
                with tc.For_i(0, n_rt, p1_unroll,
                              hint_engines=(mybir.EngineType.PE,)) as rt0:
                    p1_body(rt0)
            else:
                for rt0 in range(0, n_rt, p1_unroll):
                    p1_body(rt0)

        # ---------------- Phase 2: recurrence ----------------
        with (
            tc.tile_pool(name="wh_pool", bufs=1) as wh_pool,
            tc.tile_pool(name="xw_pool", bufs=2) as xw_pool,
            tc.tile_pool(name="gps", bufs=3, space="PSUM") as gps,
            tc.tile_pool(name="trps", bufs=2, space="PSUM") as trps,
        ):
            wh_sb = []
            for d in range(NH):
                w = wh_pool.tile([128, G4], wdt, tag=f"wh{d}")
                nc.sync.dma_start(out=w[:, :], in_=wht[128 * d:128 * (d + 1), :])
                wh_sb.append(w)

            from concourse import bass as _bass

            def step(t_iv, par):
                # gate block order [g, i, f, o]; one 2-bank psum tile per
                # block so sigmoid is a single [8,1024] ACT op that overlaps
                # the next block's matmuls.
                src_hT, dst_hT = hT_st[par], hT_st[1 - par]
                src_c, dst_c = c_st[par], c_st[1 - par]
                AL = mybir.AluOpType
                SIG = mybir.ActivationFunctionType.Sigmoid
                xw_t = xw_pool.tile([BL, G4], wdt, tag="xwt")
                nc.gpsimd.dma_start(out=xw_t[:, :], in_=xw[_bass.ts(t_iv, BL), :])
                ps_g = None
                for gb in range(4):          # g, i, f, o
                    ps = gps.tile([BL, 1024], f32, tag="gate_ps")
                    for half in range(2):
                        sl = slice(512 * half, 512 * (half + 1))
                        csl = slice(1024 * gb + 512 * half,
                                    1024 * gb + 512 * half + 512)
                        nc.tensor.matmul(ps[:, sl], ident_sb[:, :],
                                         xw_t[:, csl], start=True, stop=False)
                        for d in range(NH):
                            nc.tensor.matmul(
                                ps[:, sl],
                                src_hT[:, BL * d:BL * (d + 1)],
                                wh_sb[d][:, csl],
                                start=False, stop=(d == NH - 1),
                            )
                    if gb == 0:
                        # tanh(g) = 2*sigmoid(2g)-1; g pre-scaled by 2
                        nc.scalar.activation(ps[:, :], ps[:, :], SIG)
                        nc.vector.tensor_scalar(tg_sb[:, :], ps[:, :],
                                                2.0, -1.0, AL.mult, AL.add)
                        ps_g = ps
                    else:
                        # i/f/o: sigmoid psum -> sbuf slice
                        nc.scalar.activation(
                            sig_sb[:, 1024 * (gb - 1):1024 * gb], ps[:, :], SIG)
                sg_i = sig_sb[:, 0:H]
                sg_f = sig_sb[:, H:2 * H]
                sg_o = sig_sb[:, 2 * H:3 * H]
                nc.vector.tensor_tensor(tg_sb[:, :], sg_i, tg_sb[:, :], AL.mult)
                nc.vector.tensor_tensor(m2_sb[:, :], sg_f, src_c[:, :], AL.mult)
                nc.vector.tensor_tensor(dst_c[:, :], tg_sb[:, :], m2_sb[:, :],
                                        AL.add)
                nc.scalar.activation(th_sb[:, :], dst_c[:, :], SIG, scale=2.0)
                nc.vector.tensor_scalar(th_sb[:, :], th_sb[:, :], 2.0, -1.0,
                                        AL.mult, AL.add)
                # h + transpose per 512-half: PE transposes begin as soon as
                # the first half of h exists, shrinking the PE idle tail
                for hh in range(2):
                    hs = slice(512 * hh, 512 * (hh + 1))
                    nc.vector.tensor_tensor(h_sb[:, hs], sg_o[:, hs],
                                            th_sb[:, hs], AL.mult)
                    for d in range(4 * hh, 4 * hh + 4):
                        trp = trps.tile([128, BL], f32, tag="trp")
                        nc.tensor.transpose(
                            trp[:, :], h_sb[:, 128 * d:128 * (d + 1)],
                            identt_sb[:, :]
                        )
                        nc.vector.tensor_copy(dst_hT[:, BL * d:BL * (d + 1)],
                                              trp[:, :])
                nc.gpsimd.dma_start(out=out_h[_bass.ts(t_iv, BL), :], in_=h_sb[:, :])

            p2_unroll = min(p2_unroll, T)
            assert T % p2_unroll == 0 and p2_unroll % 2 == 0
            if use_for_i:
                with tc.For_i(0, T, p2_unroll,
                              hint_engines=(mybir.EngineType.PE,)) as t0:
                    for u in range(p2_unroll):
                        step(t0 + u, u % 2)
            else:
                for t0 in range(0, T, p2_unroll):
                    for u in range(p2_unroll):
                        step(t0 + u, u % 2)

            nc.sync.dma_start(out=out_c[:, :], in_=c_st[0][:, :])

    nc.compile()
    return nc


def _prep_core_inputs(x, W, b, mode="f32r"):
    """Host-side shard/permute. Returns list of 8 input dicts."""
    import ml_dtypes
    wdt = ml_dtypes.bfloat16 if mode == "bf16" else np.float32
    # reorder gate blocks i,f,o,g -> g,i,f,o and pre-scale g rows by 2
    # (tanh(g) computed as 2*sigmoid(2g)-1 on device)
    perm = np.concatenate([np.arange(3 * H, 4 * H), np.arange(0, 3 * H)])
    Wp = W[perm]
    bp = b[perm]
    scale = np.ones((G4,), np.float32)
    scale[:H] = 2.0
    Wx = Wp[:, :D]
    Wh = Wp[:, D:]
    wxt = np.ascontiguousarray((Wx * scale[:, None]).T).astype(wdt)   # [D, 4H]
    wht = np.ascontiguousarray((Wh * scale[:, None]).T).astype(wdt)   # [H, 4H]
    biasb = np.broadcast_to((bp * scale)[None, :], (128, G4)).astype(np.float32)
    biasb = np.ascontiguousarray(biasb)
    ident = np.eye(BL, dtype=wdt)
    identt = np.eye(BL, dtype=np.float32)
    maps = []
    for c in range(NCORES):
        xc = x[BL * c:BL * (c + 1)]                  # [8, S, D]
        xt2 = np.ascontiguousarray(xc.transpose(2, 1, 0).reshape(D, S * BL))
        maps.append({
            "xt2": xt2.astype(wdt),
            "wxt": wxt, "wht": wht, "biasb": biasb, "ident": ident,
            "identt": identt,
        })
    return maps


def _gather_outputs(results):
    outs = np.zeros((B, S, H), np.float32)
    c_fin = np.zeros((B, H), np.float32)
    for c, r in enumerate(results):
        oh = r["out_h"].reshape(S, BL, H).transpose(1, 0, 2)   # [BL, S, H]
        outs[BL * c:BL * (c + 1)] = oh
        c_fin[BL * c:BL * (c + 1)] = r["out_c"]
    h_fin = outs[:, -1, :].copy()
    return outs, (h_fin, c_fin)


def kernel(x, W, b, mode="f32r", T=S, use_for_i=True, _trace=False):
    x = np.asarray(x, np.float32)
    W = np.asarray(W, np.float32)
    b = np.asarray(b, np.float32)
    key = (T, mode, use_for_i)
    if key not in _BUILD_CACHE:
        _BUILD_CACHE[key] = _build(T=T, mode=mode, use_for_i=use_for_i)
    nc = _BUILD_CACHE[key]
    from concourse.bass_utils import run_bass_kernel_spmd
    in_maps = _prep_core_inputs(x, W, b, mode=mode)
    res = run_bass_kernel_spmd(nc, in_maps, list(range(NCORES)), trace=_trace)
    out = _gather_outputs(res.results)
    if _trace:
        return out, res
    return out
